# revision 1
# baseline (speedup 1.0000x reference)
"""Trainium2 Bass kernel for nn_MLP_Interpolate.

Reference computation (out_size=512, H=W=128, so exact 4x nearest upsample):
  out[b, :, 4k+r, 4l+s] = relu(x[b,:,k,l] @ W1[:64] + c[r,s]) @ W2 + b2
  c[r,s] = rel_y(r)*W1[64] + rel_x(s)*W1[65] + b1,  rel(t) = (2t-3)/4

Strategy (8 cores, shard = (batch, H-half)):
  - F = W1c^T x computed on PE with a block-diagonal stationary so two
    64-channel pixel groups share one pass (128 partitions fully used).
  - 16 bias+relu variants split across ACT and DVE, written into an
    interleaved rhs tile ordered by *output* column (4l+s).
  - pred on PE with block-diag [128,6] W2 stationary -> PSUM rows are
    whole contiguous output rows, DMA'd straight to DRAM.
"""

import os

import numpy as np

import concourse.bass as bass
import concourse.bacc as bacc
import concourse.mybir as mybir
import concourse.tile as tile
from concourse.bass_utils import run_bass_kernel_spmd

# Problem constants (hardcoded per contract)
B, C, H, W = 4, 64, 128, 128
OUT = 512
NF = 64  # n_feat
N_CORES = 8
ROWS_PER_CORE = H // 2          # 64 input rows per core
T_TILES = ROWS_PER_CORE // 8    # 8 F-tiles, each covering 8 input rows
REL = np.array([-0.75, -0.25, 0.25, 0.75], dtype=np.float32)

_CACHE = {}


def _build_program():
    """Build + compile the SPMD Bass program once."""
    if "nc" in _CACHE:
        return _CACHE["nc"]

    fp32 = mybir.dt.float32
    # float32r: same bytes as fp32, PE streams 1 col/cycle vs 4 for fp32
    mm_dt = (mybir.dt.float32r if os.environ.get("MM_DTYPE") == "f32r"
             else fp32)
    nc = bacc.Bacc("TRN2", target_bir_lowering=False, debug=False,
                   num_devices=N_CORES)

    x_d = nc.dram_tensor("x", [C, ROWS_PER_CORE, W], mm_dt, kind="ExternalInput")
    w1_d = nc.dram_tensor("w1diag", [128, 128], mm_dt, kind="ExternalInput")
    w2_d = nc.dram_tensor("w2diag", [128, 6], mm_dt, kind="ExternalInput")
    crs_d = nc.dram_tensor("crsT", [128, 16], fp32, kind="ExternalInput")
    out_d = nc.dram_tensor("out", [3, 4 * ROWS_PER_CORE, OUT], fp32,
                           kind="ExternalOutput")

    NT = ROWS_PER_CORE // 16  # 4 F-tiles, each 16 input rows (8 per group)

    with tile.TileContext(nc) as tc:
        with (
            tc.tile_pool(name="consts", bufs=1) as consts,
            tc.tile_pool(name="xin", bufs=2) as xin,
            tc.tile_pool(name="hbuf", bufs=2) as hbuf,
            tc.tile_pool(name="stage", bufs=6) as stage,
            tc.tile_pool(name="fpsum", bufs=2, space=bass.MemorySpace.PSUM) as fpsum,
            tc.tile_pool(name="ppsum", bufs=2, space=bass.MemorySpace.PSUM) as ppsum,
        ):
            w1_sb = consts.tile([128, 128], mm_dt)
            w2_sb = consts.tile([128, 6], mm_dt)
            crs_sb = consts.tile([128, 16], fp32)
            nc.sync.dma_start(w1_sb[:], w1_d[:])
            nc.sync.dma_start(w2_sb[:], w2_d[:])
            nc.sync.dma_start(crs_sb[:], crs_d[:])

            x_tiles = []
            f_tiles = []

            def load_x(t):
                xt = xin.tile([128, 8, W], mm_dt, tag="xt")
                # group A: rows 16t..16t+8 -> partitions 0..63 (64 channels)
                nc.sync.dma_start(xt[0:64, :, :], x_d[:, 16 * t:16 * t + 8, :])
                # group B: rows 16t+8..16t+16 -> partitions 64..127
                nc.gpsimd.dma_start(xt[64:128, :, :],
                                    x_d[:, 16 * t + 8:16 * t + 16, :])
                x_tiles.append(xt)

            def feat_matmul(t):
                ft = fpsum.tile([128, 8, W], fp32, tag="ft")
                for half in range(2):
                    nc.tensor.matmul(ft[:, 4 * half:4 * half + 4, :],
                                     w1_sb[:],
                                     x_tiles[t][:, 4 * half:4 * half + 4, :],
                                     start=True, stop=True)
                f_tiles.append(ft)

            # 10 relu variants on ACT, 6 on DVE; copies 3 ACT / 5 DVE
            ACT_V = {0, 2, 4, 6, 8, 10, 12, 14, 15, 13}

            def tile_body(t):
                ft = f_tiles[t]
                for r in range(4):
                    # h tile [part, s, i, l]: relu writes contiguous runs;
                    # the output-column interleave (4l+s) happens in the
                    # matmul rhs read AP instead (strided reads are free on
                    # PE, strided writes are ~2.7x on ACT/DVE)
                    hr = hbuf.tile([128, 4, 8, W], mm_dt, tag="hr")
                    for s in range(4):
                        v = 4 * r + s
                        bias_ap = crs_sb[:, v:v + 1]
                        if v in ACT_V:
                            nc.scalar.activation(
                                hr[:, s, :, :], ft[:, :, :],
                                mybir.ActivationFunctionType.Relu,
                                bias=bias_ap)
                        else:
                            nc.vector.tensor_scalar(
                                hr[:, s, :, :], ft[:, :, :],
                                bias_ap, 0.0,
                                mybir.AluOpType.add, mybir.AluOpType.max)

                    def mm_rhs(i):
                        # [l, s] with s innermost -> streamed col n = 4l+s
                        return hr[:, :, i, :].rearrange("p s l -> p l s")

                    copy_idx = 0
                    for ihalf in range(2):
                        if mm_dt == fp32:
                            # i-quad at (partition 32*(ii//2), slot ii%2)
                            pt = ppsum.tile([38, 2, OUT], fp32, tag="pt")
                            for ii in range(4):
                                g, j = 32 * (ii // 2), ii % 2
                                nc.tensor.matmul(pt[g:g + 6, j, :], w2_sb[:],
                                                 mm_rhs(4 * ihalf + ii),
                                                 start=True, stop=True)
                            st = stage.tile([38, 2, OUT], fp32, tag="st")
                            if (r + ihalf) % 2 == 0:
                                nc.scalar.activation(
                                    st[:, :, :], pt[:, :, :],
                                    mybir.ActivationFunctionType.Copy)
                            else:
                                nc.vector.tensor_copy(st[:, :, :],
                                                      pt[:, :, :])
                            for q in range(2):
                                for grp in range(2):
                                    row = (64 * t + 16 * ihalf + 8 * q
                                           + 32 * grp + r)
                                    eng = (nc.gpsimd if (q + grp) % 2
                                           else nc.sync)
                                    eng.dma_start(
                                        out_d[:, row:row + 5:4, :],
                                        st[32 * q + 3 * grp:
                                           32 * q + 3 * grp + 3, :, :])
                        else:
                            # f32r: matmul dst base partition must be 0
                            st = stage.tile([6, 4, OUT], fp32, tag="st")
                            for jj in range(2):
                                pt = ppsum.tile([6, 2, OUT], fp32, tag="pt")
                                for j in range(2):
                                    i = 4 * ihalf + 2 * jj + j
                                    nc.tensor.matmul(pt[:, j, :], w2_sb[:],
                                                     mm_rhs(i),
                                                     start=True, stop=True)
                                # copies: 3 on ACT, 5 on DVE per r-loop pair
                                if copy_idx in (0, 3):
                                    nc.scalar.activation(
                                        st[:, 2 * jj:2 * jj + 2, :],
                                        pt[:, :, :],
                                        mybir.ActivationFunctionType.Copy)
                                else:
                                    nc.vector.tensor_copy(
                                        st[:, 2 * jj:2 * jj + 2, :],
                                        pt[:, :, :])
                                copy_idx += 1
                            for grp in range(2):
                                row = 64 * t + 16 * ihalf + 32 * grp + r
                                eng = nc.gpsimd if grp else nc.sync
                                eng.dma_start(
                                    out_d[:, row:row + 13:4, :],
                                    st[3 * grp:3 * grp + 3, :, :])

            # software pipeline: F(t+1) issued before preds(t) so ACT/DVE
            # for tile t+1 overlap PE pred work of tile t
            load_x(0)
            feat_matmul(0)
            for t in range(NT):
                if t + 1 < NT:
                    load_x(t + 1)
                    feat_matmul(t + 1)
                tile_body(t)

    nc.compile()
    _CACHE["nc"] = nc
    return nc


def _prep_inputs(x, W1, b1, W2, b2):
    x = np.ascontiguousarray(np.asarray(x, dtype=np.float32))
    W1 = np.asarray(W1, dtype=np.float32)
    b1 = np.asarray(b1, dtype=np.float32)
    W2 = np.asarray(W2, dtype=np.float32)

    w1c = W1[:NF]                      # [64, 64]
    w1diag = np.zeros((128, 128), dtype=np.float32)
    w1diag[0:64, 0:64] = w1c
    w1diag[64:128, 64:128] = w1c

    w2diag = np.zeros((128, 6), dtype=np.float32)
    w2diag[0:64, 0:3] = W2
    w2diag[64:128, 3:6] = W2

    # c[r,s] = rel[r]*W1[64] + rel[s]*W1[65] + b1 -> [16, 64]
    crs = (REL[:, None, None] * W1[NF][None, None, :]
           + REL[None, :, None] * W1[NF + 1][None, None, :]
           + b1[None, None, :]).reshape(16, NF)
    crsT = np.ascontiguousarray(
        np.concatenate([crs.T, crs.T], axis=0))  # [128, 16]

    in_maps = []
    for c in range(N_CORES):
        b, half = c // 2, c % 2
        xs = np.ascontiguousarray(
            x[b, :, half * ROWS_PER_CORE:(half + 1) * ROWS_PER_CORE, :])
        in_maps.append({"x": xs, "w1diag": w1diag, "w2diag": w2diag,
                        "crsT": crsT})
    return in_maps


def _gather(results, b2):
    full = np.empty((B, 3, OUT, OUT), dtype=np.float32)
    for c in range(N_CORES):
        b, half = c // 2, c % 2
        full[b, :, half * (OUT // 2):(half + 1) * (OUT // 2), :] = \
            results[c]["out"]
    b2 = np.asarray(b2, dtype=np.float32)
    if np.any(b2):
        full += b2.reshape(1, 3, 1, 1)
    return full


def run(trace=False, **inputs):
    nc = _build_program()
    in_maps = _prep_inputs(inputs["x"], inputs["W1"], inputs["b1"],
                           inputs["W2"], inputs["b2"])
    res = run_bass_kernel_spmd(nc, in_maps, list(range(N_CORES)), trace=trace)
    return _gather(res.results, inputs["b2"]), res


def kernel(**inputs):
    out, _ = run(trace=False, **inputs)
    return out



# revision 4
# speedup vs baseline: 1.9584x; 1.9584x over previous
"""Trainium2 Bass kernel for nn_MLP_Interpolate.

Reference computation (out_size=512, H=W=128 -> exact 4x nearest upsample):
  out[b, :, 4i+r, 4l+s] = relu(x[b,:,i,l] @ W1[:64] + c[r,s]) @ W2 + b2
  c[r,s] = rel_y(r)*W1[64] + rel_x(s)*W1[65] + b1,  rel(t) = (2t-3)/4

Strategy (8 cores, shard = (batch, H-half)), all-bf16 datapath:
  - stage 1 (features F = W1c^T x): one bf16 matmul per 1024 pixels with a
    block-diagonal [128,128] stationary so two 64-channel pixel groups
    (A = top half rows, B = bottom half) share one pass.
  - ACT copies F (PSUM fp32) -> bf16 SBUF; DVE then computes all 16
    bias+relu variants as bf16 tensor_scalar ops (4x perf mode).
  - stage 2 (pred = h @ W2): [128,6] block-diag W2 stationary placed on 4
    independent PE column strips via tile_position=(0,32r) so 4 matmuls
    stream concurrently. Moving operand reads h with the (l,s) interleave
    so PSUM rows are whole output rows.
  - ACT copies pred (PSUM) -> bf16 staging; two DMA queues (sync/gpsimd)
    write 48KB batched descriptors straight to DRAM. Host upcasts + b2.
"""

import numpy as np
import ml_dtypes

import concourse.bass as bass
import concourse.bacc as bacc
import concourse.mybir as mybir
import concourse.tile as tile
from concourse.bass_utils import run_bass_kernel_spmd

# Problem constants (hardcoded per contract)
B, C, H, W = 4, 64, 128, 128
OUT = 512
NF = 64
N_CORES = 8
ROWS_PER_CORE = H // 2          # 64 input rows per core
NBLK = 2                        # blocks per core; block = 16 A-rows + 16 B-rows
BI = 16                         # input rows per group per block
REL = np.array([-0.75, -0.25, 0.25, 0.75], dtype=np.float32)

_CACHE = {}


def _build_program():
    if "nc" in _CACHE:
        return _CACHE["nc"]

    fp32 = mybir.dt.float32
    bf16 = mybir.dt.bfloat16
    nc = bacc.Bacc("TRN2", target_bir_lowering=False, debug=False,
                   num_devices=N_CORES)

    x_d = nc.dram_tensor("x", [C, ROWS_PER_CORE, W], bf16, kind="ExternalInput")
    w1_d = nc.dram_tensor("w1diag", [128, 128], bf16, kind="ExternalInput")
    w2_d = nc.dram_tensor("w2diag", [128, 6], bf16, kind="ExternalInput")
    crs_d = nc.dram_tensor("crsT", [128, 16], fp32, kind="ExternalInput")
    out_d = nc.dram_tensor("out", [3, 4 * ROWS_PER_CORE, OUT], bf16,
                           kind="ExternalOutput")

    with tile.TileContext(nc) as tc:
        with (
            tc.tile_pool(name="consts", bufs=1) as consts,
            tc.tile_pool(name="xin", bufs=2) as xin,
            tc.tile_pool(name="fbf", bufs=2) as fbfp,
            tc.tile_pool(name="hbuf", bufs=2) as hbuf,
            tc.tile_pool(name="stage", bufs=2) as stage,
            tc.tile_pool(name="fpsum", bufs=2, space=bass.MemorySpace.PSUM) as fpsum,
            tc.tile_pool(name="ppsum", bufs=2, space=bass.MemorySpace.PSUM) as ppsum,
        ):
            w1_sb = consts.tile([128, 128], bf16)
            w2_sb = consts.tile([128, 6], bf16)
            crs_sb = consts.tile([128, 16], fp32)
            nc.sync.dma_start(w1_sb[:], w1_d[:])
            nc.sync.dma_start(w2_sb[:], w2_d[:])
            nc.sync.dma_start(crs_sb[:], crs_d[:])

            x_tiles, f_tiles, fbf_tiles = [], [], []

            def load_x(b):
                xt = xin.tile([128, BI, W], bf16, tag="xt")
                nc.sync.dma_start(xt[0:64, :, :],
                                  x_d[:, BI * b:BI * (b + 1), :])
                nc.gpsimd.dma_start(xt[64:128, :, :],
                                    x_d[:, 32 + BI * b:32 + BI * (b + 1), :])
                x_tiles.append(xt)

            def feat(b):
                # two 1024-col stage-1 matmuls -> F halves in PSUM
                xt = x_tiles[b]
                fs = []
                for half in range(2):
                    ft = fpsum.tile([128, 8, W], fp32, tag="ft")
                    for q in range(2):
                        sl = slice(4 * q, 4 * q + 4)
                        nc.tensor.matmul(ft[:, sl, :], w1_sb[:],
                                         xt[:, 8 * half:8 * half + 8, :][:, sl, :],
                                         start=True, stop=True)
                    fs.append(ft)
                f_tiles.append(fs)

            def fbf_copy(b):
                # ACT: PSUM fp32 -> SBUF bf16 (feeds DVE 4x relu)
                fb = fbfp.tile([128, 2 * 8 * W], bf16, tag="fb")
                for half in range(2):
                    nc.scalar.activation(
                        fb[:, 1024 * half:1024 * (half + 1)],
                        f_tiles[b][half][:, :, :],
                        mybir.ActivationFunctionType.Copy)
                fbf_tiles.append(fb)

            def body(b):
                fb = fbf_tiles[b]
                # DVE: 16 bias+relu variants, each [128, 2048] bf16 (4x mode)
                h = hbuf.tile([128, 16, 2048], bf16, tag="h")
                for v in range(16):
                    nc.vector.tensor_scalar(
                        h[:, v, :], fb[:, :],
                        crs_sb[:, v:v + 1], 0.0,
                        mybir.AluOpType.add, mybir.AluOpType.max)

                # stage 2: per i-pair, 8 matmuls on 4 PE column strips
                st = stage.tile([102, BI, OUT], bf16, tag="st")
                for m in range(BI // 2):
                    pt = ppsum.tile([102, 2, OUT], fp32, tag="pt")
                    for j in range(2):
                        i = 2 * m + j
                        for r in range(4):
                            rhs = h[:, 4 * r:4 * r + 4,
                                    128 * i:128 * (i + 1)].rearrange(
                                        "p s l -> p l s")
                            nc.tensor.matmul(pt[32 * r:32 * r + 6, j, :],
                                             w2_sb[:], rhs,
                                             start=True, stop=True,
                                             tile_position=(0, 32 * r))
                    nc.scalar.activation(
                        st[:, 2 * m:2 * m + 2, :], pt[:, :, :],
                        mybir.ActivationFunctionType.Copy)

                # 8 batched output DMAs: [3 part, 16 rows(stride 4), 512]
                for r in range(4):
                    for ab in range(2):
                        row0 = 128 * ab + 4 * BI * b + r
                        eng = nc.gpsimd if (r + ab) % 2 else nc.sync
                        eng.dma_start(
                            out_d[:, row0:row0 + 4 * BI - 3:4, :],
                            st[32 * r + 3 * ab:32 * r + 3 * ab + 3, :, :])

            load_x(0)
            feat(0)
            fbf_copy(0)
            for b in range(NBLK):
                if b + 1 < NBLK:
                    load_x(b + 1)
                    feat(b + 1)
                    fbf_copy(b + 1)
                body(b)

    nc.compile()
    _CACHE["nc"] = nc
    return nc


def _prep_inputs(x, W1, b1, W2, b2):
    x = np.asarray(x, dtype=np.float32)
    W1 = np.asarray(W1, dtype=np.float32)
    b1 = np.asarray(b1, dtype=np.float32)
    W2 = np.asarray(W2, dtype=np.float32)

    w1c = W1[:NF]
    w1diag = np.zeros((128, 128), dtype=np.float32)
    w1diag[0:64, 0:64] = w1c
    w1diag[64:128, 64:128] = w1c

    w2diag = np.zeros((128, 6), dtype=np.float32)
    w2diag[0:64, 0:3] = W2
    w2diag[64:128, 3:6] = W2

    # c[r,s] = rel[r]*W1[64] + rel[s]*W1[65] + b1 -> [16, 64] -> [128, 16]
    crs = (REL[:, None, None] * W1[NF][None, None, :]
           + REL[None, :, None] * W1[NF + 1][None, None, :]
           + b1[None, None, :]).reshape(16, NF)
    crsT = np.ascontiguousarray(np.concatenate([crs.T, crs.T], axis=0))

    w1_bf = w1diag.astype(ml_dtypes.bfloat16)
    w2_bf = w2diag.astype(ml_dtypes.bfloat16)

    in_maps = []
    for c in range(N_CORES):
        b, half = c // 2, c % 2
        xs = np.ascontiguousarray(
            x[b, :, half * ROWS_PER_CORE:(half + 1) * ROWS_PER_CORE, :]
        ).astype(ml_dtypes.bfloat16)
        in_maps.append({"x": xs, "w1diag": w1_bf, "w2diag": w2_bf,
                        "crsT": crsT})
    return in_maps


def _gather(results, b2):
    full = np.empty((B, 3, OUT, OUT), dtype=np.float32)
    for c in range(N_CORES):
        b, half = c // 2, c % 2
        full[b, :, half * (OUT // 2):(half + 1) * (OUT // 2), :] = \
            np.asarray(results[c]["out"]).astype(np.float32)
    b2 = np.asarray(b2, dtype=np.float32)
    if np.any(b2):
        full += b2.reshape(1, 3, 1, 1)
    return full


def run(trace=False, **inputs):
    nc = _build_program()
    in_maps = _prep_inputs(inputs["x"], inputs["W1"], inputs["b1"],
                           inputs["W2"], inputs["b2"])
    res = run_bass_kernel_spmd(nc, in_maps, list(range(N_CORES)), trace=trace)
    return _gather(res.results, inputs["b2"]), res


def kernel(**inputs):
    out, _ = run(trace=False, **inputs)
    return out


# revision 5
# speedup vs baseline: 2.3206x; 1.1850x over previous
"""Trainium2 Bass kernel for nn_MLP_Interpolate.

Reference computation (out_size=512, H=W=128 -> exact 4x nearest upsample):
  out[b, :, 4i+r, 4l+s] = relu(x[b,:,i,l] @ W1[:64] + c[r,s]) @ W2 + b2
  c[r,s] = rel_y(r)*W1[64] + rel_x(s)*W1[65] + b1,  rel(t) = (2t-3)/4

Strategy (8 cores, shard = (batch, H-half)), all-bf16 datapath:
  - stage 1 (features F = W1c^T x): one bf16 matmul per 1024 pixels with a
    block-diagonal [128,128] stationary so two 64-channel pixel groups
    (A = top half rows, B = bottom half) share one pass.
  - ACT copies F (PSUM fp32) -> bf16 SBUF; DVE then computes all 16
    bias+relu variants as bf16 tensor_scalar ops (4x perf mode).
  - stage 2 (pred = h @ W2): [128,6] block-diag W2 stationary placed on 4
    independent PE column strips via tile_position=(0,32r) so 4 matmuls
    stream concurrently. Moving operand reads h with the (l,s) interleave
    so PSUM rows are whole output rows.
  - ACT copies pred (PSUM) -> bf16 staging; two DMA queues (sync/gpsimd)
    write 48KB batched descriptors straight to DRAM. Host upcasts + b2.
"""

import numpy as np
import ml_dtypes

import concourse.bass as bass
import concourse.bacc as bacc
import concourse.mybir as mybir
import concourse.tile as tile
from concourse.bass_utils import run_bass_kernel_spmd

# Problem constants (hardcoded per contract)
B, C, H, W = 4, 64, 128, 128
OUT = 512
NF = 64
N_CORES = 8
ROWS_PER_CORE = H // 2          # 64 input rows per core
NBLK = 2                        # blocks per core; block = 16 A-rows + 16 B-rows
BI = 16                         # input rows per group per block
REL = np.array([-0.75, -0.25, 0.25, 0.75], dtype=np.float32)

_CACHE = {}


def _build_program():
    if "nc" in _CACHE:
        return _CACHE["nc"]

    fp32 = mybir.dt.float32
    bf16 = mybir.dt.bfloat16
    nc = bacc.Bacc("TRN2", target_bir_lowering=False, debug=False,
                   num_devices=N_CORES)

    x_d = nc.dram_tensor("x", [C, ROWS_PER_CORE, W], bf16, kind="ExternalInput")
    w1_d = nc.dram_tensor("w1diag", [128, 128], bf16, kind="ExternalInput")
    w2_d = nc.dram_tensor("w2diag", [128, 6], bf16, kind="ExternalInput")
    crs_d = nc.dram_tensor("crsT", [128, 16], fp32, kind="ExternalInput")
    out_d = nc.dram_tensor("out", [3, 4 * ROWS_PER_CORE, OUT], bf16,
                           kind="ExternalOutput")

    with tile.TileContext(nc) as tc:
        with (
            tc.tile_pool(name="consts", bufs=1) as consts,
            tc.tile_pool(name="xin", bufs=2) as xin,
            tc.tile_pool(name="fbf", bufs=2) as fbfp,
            tc.tile_pool(name="hbuf", bufs=2) as hbuf,
            tc.tile_pool(name="stage", bufs=2) as stage,
            tc.tile_pool(name="fpsum", bufs=2, space=bass.MemorySpace.PSUM) as fpsum,
            tc.tile_pool(name="ppsum", bufs=2, space=bass.MemorySpace.PSUM) as ppsum,
        ):
            w1_sb = consts.tile([128, 128], bf16)
            w2_sb = consts.tile([128, 6], bf16)
            crs_sb = consts.tile([128, 16], fp32)
            nc.sync.dma_start(w1_sb[:], w1_d[:])
            nc.sync.dma_start(w2_sb[:], w2_d[:])
            nc.sync.dma_start(crs_sb[:], crs_d[:])

            x_tiles, f_tiles, fbf_tiles = [], [], []

            def load_x(b):
                xt = xin.tile([128, BI, W], bf16, tag="xt")
                nc.sync.dma_start(xt[0:64, :, :],
                                  x_d[:, BI * b:BI * (b + 1), :])
                nc.gpsimd.dma_start(xt[64:128, :, :],
                                    x_d[:, 32 + BI * b:32 + BI * (b + 1), :])
                x_tiles.append(xt)

            def feat(b):
                # two 1024-col stage-1 matmuls -> F halves in PSUM
                xt = x_tiles[b]
                fs = []
                for half in range(2):
                    ft = fpsum.tile([128, 8, W], fp32, tag="ft")
                    for q in range(2):
                        sl = slice(4 * q, 4 * q + 4)
                        nc.tensor.matmul(ft[:, sl, :], w1_sb[:],
                                         xt[:, 8 * half:8 * half + 8, :][:, sl, :],
                                         start=True, stop=True)
                    fs.append(ft)
                f_tiles.append(fs)

            def fbf_copy(b):
                # ACT: PSUM fp32 -> SBUF bf16 (feeds DVE 4x relu)
                fb = fbfp.tile([128, 2 * 8 * W], bf16, tag="fb")
                for half in range(2):
                    nc.scalar.activation(
                        fb[:, 1024 * half:1024 * (half + 1)],
                        f_tiles[b][half][:, :, :],
                        mybir.ActivationFunctionType.Copy)
                fbf_tiles.append(fb)

            def body(b):
                fb = fbf_tiles[b]
                # DVE: 16 bias+relu variants, each [128, 2048] bf16 (4x mode)
                h = hbuf.tile([128, 16, 2048], bf16, tag="h")
                for v in range(16):
                    nc.vector.tensor_scalar(
                        h[:, v, :], fb[:, :],
                        crs_sb[:, v:v + 1], 0.0,
                        mybir.AluOpType.add, mybir.AluOpType.max)

                # stage 2: per i-pair, 8 matmuls on 4 PE column strips
                st = stage.tile([102, BI, OUT], bf16, tag="st")
                for m in range(BI // 2):
                    pt = ppsum.tile([102, 2, OUT], fp32, tag="pt")
                    for j in range(2):
                        i = 2 * m + j
                        for r in range(4):
                            # natural order: l contiguous innermost -> PE
                            # streams at full rate; PSUM cols are s*128+l
                            rhs = h[:, 4 * r:4 * r + 4, 128 * i:128 * (i + 1)]
                            nc.tensor.matmul(pt[32 * r:32 * r + 6, j, :],
                                             w2_sb[:], rhs,
                                             start=True, stop=True,
                                             tile_position=(0, 32 * r))
                    # un-interleave (s,l) -> (4l+s) via strided PSUM read
                    nc.scalar.activation(
                        st[:, 2 * m:2 * m + 2, :],
                        pt[:, :, :].rearrange("p j (s l) -> p j l s", s=4),
                        mybir.ActivationFunctionType.Copy)

                # 8 batched output DMAs: [3 part, 16 rows(stride 4), 512]
                for r in range(4):
                    for ab in range(2):
                        row0 = 128 * ab + 4 * BI * b + r
                        eng = nc.gpsimd if (r + ab) % 2 else nc.sync
                        eng.dma_start(
                            out_d[:, row0:row0 + 4 * BI - 3:4, :],
                            st[32 * r + 3 * ab:32 * r + 3 * ab + 3, :, :])

            load_x(0)
            feat(0)
            fbf_copy(0)
            for b in range(NBLK):
                if b + 1 < NBLK:
                    load_x(b + 1)
                    feat(b + 1)
                    fbf_copy(b + 1)
                body(b)

    nc.compile()
    _CACHE["nc"] = nc
    return nc


def _prep_inputs(x, W1, b1, W2, b2):
    x = np.asarray(x, dtype=np.float32)
    W1 = np.asarray(W1, dtype=np.float32)
    b1 = np.asarray(b1, dtype=np.float32)
    W2 = np.asarray(W2, dtype=np.float32)

    w1c = W1[:NF]
    w1diag = np.zeros((128, 128), dtype=np.float32)
    w1diag[0:64, 0:64] = w1c
    w1diag[64:128, 64:128] = w1c

    w2diag = np.zeros((128, 6), dtype=np.float32)
    w2diag[0:64, 0:3] = W2
    w2diag[64:128, 3:6] = W2

    # c[r,s] = rel[r]*W1[64] + rel[s]*W1[65] + b1 -> [16, 64] -> [128, 16]
    crs = (REL[:, None, None] * W1[NF][None, None, :]
           + REL[None, :, None] * W1[NF + 1][None, None, :]
           + b1[None, None, :]).reshape(16, NF)
    crsT = np.ascontiguousarray(np.concatenate([crs.T, crs.T], axis=0))

    w1_bf = w1diag.astype(ml_dtypes.bfloat16)
    w2_bf = w2diag.astype(ml_dtypes.bfloat16)

    in_maps = []
    for c in range(N_CORES):
        b, half = c // 2, c % 2
        xs = np.ascontiguousarray(
            x[b, :, half * ROWS_PER_CORE:(half + 1) * ROWS_PER_CORE, :]
        ).astype(ml_dtypes.bfloat16)
        in_maps.append({"x": xs, "w1diag": w1_bf, "w2diag": w2_bf,
                        "crsT": crsT})
    return in_maps


def _gather(results, b2):
    full = np.empty((B, 3, OUT, OUT), dtype=np.float32)
    for c in range(N_CORES):
        b, half = c // 2, c % 2
        full[b, :, half * (OUT // 2):(half + 1) * (OUT // 2), :] = \
            np.asarray(results[c]["out"]).astype(np.float32)
    b2 = np.asarray(b2, dtype=np.float32)
    if np.any(b2):
        full += b2.reshape(1, 3, 1, 1)
    return full


def run(trace=False, **inputs):
    nc = _build_program()
    in_maps = _prep_inputs(inputs["x"], inputs["W1"], inputs["b1"],
                           inputs["W2"], inputs["b2"])
    res = run_bass_kernel_spmd(nc, in_maps, list(range(N_CORES)), trace=trace)
    return _gather(res.results, inputs["b2"]), res


def kernel(**inputs):
    out, _ = run(trace=False, **inputs)
    return out
